# revision 28
# baseline (speedup 1.0000x reference)
"""Trainium2 Bass kernel for BidPrefix: per-row cumprod + 3-point gather.

Reference semantics (per row b of inputs [B, 302]):
  rates = inputs[b, :300]; bid = int(inputs[b, 300]); mp = int(inputs[b, 301])
  cpz[k] = prod(rates[:k]) (cpz[0] = 1)
  out[b] = [cpz[bid], cpz[mp+1], cpz[mp]]

Strategy: pure data parallel over 8 NeuronCores. Rows are host-sorted by
max(bid, mp) descending and packed 128-per-tile so every tap in tile t
lies below a per-tile bound L[t]. Tiles are batched into groups; per
group the host packs three contiguous header blocks (bid[gsz], mp[gsz],
rates[mp][gsz]) followed by fully contiguous per-(partition, tile) pages
[1.0, rates[0:W]] (W = group max L) in a flat [128, TOT] DRAM layout, so
the per-group DMA is one contiguous slab, only ~2/3 of the rate columns
ever move, and the DVE streams gap-free pages (strided pages cost ~25%
extra per element).

On device, one custom DVE op (PAGETAP_ANT) processes a whole group per
instruction: a 3-state uop FSM (seed / steady / page-step) runs, per
page, pgidx = 0,1,2,..., cp = running product of the streamed page
(which starts with the packed 1.0, so cp[e] = cpz[e] exactly), and an
accumulator R += (pgidx == tap) * cp that is re-seeded at each page
boundary by the hand-written step uop. R rides the BYPASS chain to the
write port with a stride-0 output AP, so the last element written per
page is cpz[tap]. Two passes per group (tap = bid, tap = mp) give
cpz[bid] and cpz[mp]; cpz[mp+1] = cpz[mp] * rates[mp] is one small
GpSimd multiply per group against the packed rates[mp] column (bit-exact
with the reference's sequential f32 cumprod). The leading-1.0 trick
makes bid==0 / mp==0 fall out naturally (cp[0] = 1), so there are no
edge-case patches. The host does layout only (sort, pad, duplicate
rates[mp] into the page header); every multiply happens on device.
"""

import dataclasses
import sys

if "/opt/trn_rl_repo" not in sys.path:
    sys.path.insert(0, "/opt/trn_rl_repo")

import numpy as np

S = 300
COLS = 302
P = 128
NCORES = 8
TILES = 196
BPC = TILES * P  # 25088 rows per core
BTOT = 200000
HDR = 4  # page header: bid, mp, rates[mp], 1.0

TRACE = False
LAST_RESULTS = None

_PAGETAP = None


def _get_pagetap():
    """Register the batched page-tap custom DVE op (idempotent).

    For in0 = [P, S, N] pages x and in1 = per-page tap index t (stride-0
    broadcast), each page computes R_e = sum_{k<=e} [k == t] * cumprod(x)[k]
    with cumprod and R reset at every page boundary; out (stride-0 per
    page) keeps R_N-1 = cumprod(x)[t].
    """
    global _PAGETAP
    if _PAGETAP is not None:
        return _PAGETAP
    import concourse.dve_ops as dve_ops
    from concourse.dve_ops import OPS, DveOp
    from concourse.dve_spec import (
        AluOp, Bin, Latch, Scan, Spec, Src0, Src1, Zero, One, eq,
        _assemble, _build_placement, _build_state_machine, _collect,
        _hoist_stream_invariant_ops, _validate_body, _Stage, PREV,
    )
    from concourse.dve_uop import (
        DveOpSpec, Trigger, OutSel, OutPath, ENABLE, N_LANES, N_STAGES,
    )

    name = "PAGETAP_ANT"
    for op in OPS:
        if op.name == name:
            _PAGETAP = op
            return op

    def _ref(in0, in1, s0, s1, imm2):
        x = in0.astype(np.float32)
        n = x.shape[-1]
        cp = np.cumprod(x, axis=-1, dtype=np.float32)
        tap = np.asarray(in1, np.float32)[..., :1]
        idxs = np.arange(n, dtype=np.float32)
        run = np.cumsum((idxs == tap) * cp, axis=-1, dtype=np.float32)
        return run

    pgidx = Scan(AluOp.ADD, One, init=Bin(AluOp.SUBTRACT, Zero, One))
    cps = Scan(AluOp.MULTIPLY, Src0, init=One)
    spec = Spec(
        body=eq(pgidx, Src1) * cps,
        accum=AluOp.ADD,
        accum_init=Zero,
        reference=_ref,
    )

    def _uops(ver):
        _validate_body(spec, ver)
        spec2 = _hoist_stream_invariant_ops(spec)
        scans = _collect(spec2.body, Scan)
        latches = _collect(spec2.body, Latch)
        p = _build_placement(spec2, scans, N_STAGES[ver], N_LANES[ver])
        states = _build_state_machine(spec2, scans, latches, p)
        assert len(states) == 2, states
        seed, steady = states
        pg2 = [s for s in scans if s.op == AluOp.ADD][0]
        cp2 = [s for s in scans if s.op == AluOp.MULTIPLY][0]
        steady2 = dataclasses.replace(
            steady,
            trigger=(Trigger.SRC_TENSOR_DONE, Trigger.SUB_DIM_DONE, Trigger.NONE),
            next=(0, 2, 0),
        )
        # page-boundary step uop: processes the first element of the new
        # page with the two scans re-seeded (pgidx <- 0, cp <- x) and the
        # accumulator restarted (R <- 0 + body)
        ov = {
            p.node_stage[pg2]: _Stage(AluOp.BYPASS, Zero),
            p.node_stage[cp2]: _Stage(AluOp.BYPASS, Src0),
            p.accum_stage: _Stage(AluOp.ADD, Zero, PREV),
        }
        step = dataclasses.replace(
            steady,
            overrides=ov,
            trigger=(Trigger.SRC_TENSOR_DONE, Trigger.SUB_DIM_DONE, Trigger.COUNT),
            next=(0, 2, 1),
            repeat=1,
        )
        uops = [_assemble(st) for st in (seed, steady2, step)]
        # the running sum rides the BYPASS chain to block 7's ALU_OUT;
        # write it every element (stride-0 out AP keeps the page-final one)
        for u in uops[1:]:
            u.out[OutPath.WR0_LO] = OutSel.ALU_OUT
            u.out_enable[OutPath.WR0_LO] = ENABLE
        return uops

    raw = {ver: _uops(ver) for ver in ("v3", "v4")}

    @dataclasses.dataclass(frozen=True)
    class _RawDveOp(DveOp):
        raw_uops: dict = dataclasses.field(
            default_factory=dict, compare=False, hash=False
        )

        def compile(self, ver):
            sp = DveOpSpec(
                name=self.name,
                opcode=dve_ops.get_dve_sub_opcode(self.name),
                uops=self.raw_uops[ver],
                rd1_en=True,
            )
            sp.validate(ver)
            return sp

    shas = {
        ver: DveOpSpec(name=name, opcode=0, uops=u, rd1_en=True).sha(ver)
        for ver, u in raw.items()
    }
    op = _RawDveOp(name, spec, subdim=True, uops_sha=shas, raw_uops=raw)
    OPS.append(op)
    dve_ops._SUB_OPCODE_FOR_NAME[name] = (
        dve_ops._CUSTOM_DVE_ROW_BASE + len(OPS) - 1
    )
    dve_ops.CUSTOM_DVE_SPECS[name] = spec
    _PAGETAP = op
    return op


def _plan_groups(L_list):
    """Greedy tile grouping: per group, page width = W+1 where W = max L
    in the group (tiles arrive sorted desc, so W = L[t0]); fill until the
    per-partition element budget is hit. Small ramp-up budgets let the DVE
    start before a full-size DMA lands; a small tail shortens the drain."""
    n = len(L_list)
    budgets = [384, 768, 1536, 3072] + [6144] * n
    groups = []
    t0 = 0
    gi = 0
    while t0 < n:
        budget = budgets[min(gi, len(budgets) - 1)]
        W = max(int(L_list[t0]), 1)
        gsz = max(1, budget // (W + HDR))
        gsz = min(gsz, n - t0)
        rem = n - t0 - gsz
        if 0 < rem < 3:
            gsz = max(1, gsz - (3 - rem))
        groups.append((t0, gsz, max(int(L_list[t0]), 1)))
        t0 += gsz
        gi += 1
    # split the final group into a ramp-down if it is large
    t0, gsz, W = groups[-1]
    if gsz >= 12:
        groups[-1] = (t0, gsz - 8, W)
        groups.append((t0 + gsz - 8, 6, max(int(L_list[t0 + gsz - 8]), 1)))
        groups.append((t0 + gsz - 2, 2, max(int(L_list[t0 + gsz - 2]), 1)))
    return groups


def _group_cols(gsz, W):
    """Per-partition f32 slots for one group: 3 header blocks (bid, mp,
    rates[mp]; each [gsz]) + contiguous rate pages [gsz, W+1] (leading 1.0
    + W rates)."""
    return 3 * gsz + gsz * (W + 1)


def build_nc(L_list, groups=None):
    import concourse.bacc as bacc
    import concourse.mybir as mybir
    from concourse import tile

    f32 = mybir.dt.float32
    A = mybir.AluOpType
    TAP = _get_pagetap()

    if groups is None:
        groups = _plan_groups(L_list)
    ntiles = len(L_list)
    offs = [0]
    for _, gsz, W in groups:
        offs.append(offs[-1] + _group_cols(gsz, W))
    TOT = offs[-1]

    nc = bacc.Bacc("TRN2", target_bir_lowering=False, debug=False)
    inp = nc.dram_tensor("inp", [P, TOT], f32, kind="ExternalInput")
    out = nc.dram_tensor("out", [P, ntiles * 3], f32, kind="ExternalOutput")
    vin = inp.ap()
    vout = out.ap()

    with tile.TileContext(nc) as tc:
        with (
            tc.tile_pool(name="raw", bufs=5) as rawp,
            tc.tile_pool(name="res", bufs=5) as resp,
        ):
            prepped = {}

            def prep(gj):
                _, gsz, W = groups[gj]
                g = rawp.tile([P, _group_cols(gsz, W)], f32, tag="raw")
                nc.sync.dma_start(g, vin[:, offs[gj] : offs[gj + 1]])
                prepped[gj] = g

            for gj in range(min(4, len(groups))):
                prep(gj)
            for gi, (t0, gsz, W) in enumerate(groups):
                if gi + 4 < len(groups):
                    prep(gi + 4)
                N = W + 1
                g = prepped.pop(gi)
                rates = g[:, 3 * gsz :].rearrange("p (s w) -> p s w", w=N)
                # res layout [P, 3, gsz]: k-major so every operand below is
                # a contiguous [P, gsz] block (strided APs are slow on the
                # Q7 gpsimd engine)
                res = resp.tile([P, 3 * gsz], f32, tag="res")

                def col(ap, j0, n=gsz):
                    return ap[:, j0 * gsz : j0 * gsz + n]

                nc.vector._custom_dve(
                    TAP,
                    out=col(res, 0).unsqueeze(2).broadcast_to([P, gsz, N]),
                    in0=rates,
                    in1=col(g, 0).unsqueeze(2).broadcast_to([P, gsz, N]),
                )
                nc.vector._custom_dve(
                    TAP,
                    out=col(res, 2).unsqueeze(2).broadcast_to([P, gsz, N]),
                    in0=rates,
                    in1=col(g, 1).unsqueeze(2).broadcast_to([P, gsz, N]),
                )
                # cpz[mp+1] = cpz[mp] * rates[mp] (packed header block) on
                # the otherwise-idle GpSimd engine, overlapped with the next
                # group's DVE passes
                nc.gpsimd.tensor_tensor(
                    col(res, 1), col(res, 2), col(g, 2), A.mult
                )
                # result DMAs ride the Activation HWDGE queue so the Sync
                # queue only carries (latency-critical) input-slab issues
                nc.scalar.dma_start(vout[:, t0 * 3 : (t0 + gsz) * 3], res)

    nc.compile()
    return nc


def _prepare(x, ncores, tiles):
    """Sort rows by max(bid, mp) desc, pack into per-core flat page layout.

    Returns (arrs [ncores, P, TOT], L_list, groups, src_cpt)."""
    bpc = tiles * P
    npad = bpc * ncores - x.shape[0]
    assert npad >= 0
    if npad:
        padrows = np.zeros((npad, COLS), dtype=np.float32)
        padrows[:, :S] = 1.0
        xp = np.concatenate([x, padrows], axis=0)
    else:
        xp = x

    key = np.maximum(xp[:, S], xp[:, S + 1]).astype(np.int64)
    order = np.argsort(-key, kind="stable")
    nblocks = ncores * tiles
    src = order.reshape(nblocks, P).reshape(tiles, ncores, P)
    src_cpt = np.ascontiguousarray(src.transpose(1, 2, 0))  # [core, p, t]

    block_max = key[order].reshape(nblocks, P)[:, 0]
    L_list = np.maximum(block_max.reshape(tiles, ncores).max(axis=1), 1)
    L_list = [int(v) for v in L_list]
    groups = _plan_groups(L_list)

    rows = xp[src_cpt]  # [ncores, P, tiles, COLS]
    parts = []
    for t0, gsz, W in groups:
        rg = rows[:, :, t0 : t0 + gsz, :]
        hdr = np.empty((ncores, P, 3, gsz), dtype=np.float32)
        hdr[:, :, 0] = rg[..., S]
        hdr[:, :, 1] = rg[..., S + 1]
        mp_i = rg[..., S + 1].astype(np.int64)[..., None]
        hdr[:, :, 2] = np.take_along_axis(rg[..., :S], mp_i, axis=-1)[..., 0]
        pg = np.empty((ncores, P, gsz, W + 1), dtype=np.float32)
        pg[..., 0] = 1.0
        pg[..., 1:] = rg[..., :W]
        parts.append(hdr.reshape(ncores, P, 3 * gsz))
        parts.append(pg.reshape(ncores, P, gsz * (W + 1)))
    arrs = np.concatenate(parts, axis=2)
    return np.ascontiguousarray(arrs), L_list, groups, src_cpt


_NC_CACHE = {}


def _get_nc(L_list, groups):
    key = tuple(L_list)
    if key not in _NC_CACHE:
        _NC_CACHE[key] = build_nc(L_list, groups)
    return _NC_CACHE[key]


def kernel(inputs):
    global LAST_RESULTS
    x = np.ascontiguousarray(np.asarray(inputs), dtype=np.float32)
    assert x.shape == (BTOT, COLS), x.shape

    arrs, L_list, groups, src_cpt = _prepare(x, NCORES, TILES)
    in_maps = [{"inp": np.ascontiguousarray(arrs[c])} for c in range(NCORES)]

    nc = _get_nc(L_list, groups)
    from concourse.bass_utils import run_bass_kernel_spmd

    r = run_bass_kernel_spmd(
        nc, in_maps, core_ids=list(range(NCORES)), trace=TRACE
    )
    LAST_RESULTS = r
    ys = np.empty((NCORES, P, TILES, 3), dtype=np.float32)
    for c in range(NCORES):
        yc = np.asarray(r.results[c]["out"])  # [P, 3*TILES], k-major blocks
        for t0, gsz, W in groups:
            blk = yc[:, 3 * t0 : 3 * (t0 + gsz)].reshape(P, 3, gsz)
            ys[c, :, t0 : t0 + gsz, :] = blk.transpose(0, 2, 1)
    out = np.empty((NCORES * BPC, 3), dtype=np.float32)
    out[src_cpt.reshape(-1)] = ys.reshape(-1, 3)
    return np.ascontiguousarray(out[:BTOT])
